# revision 44
# baseline (speedup 1.0000x reference)
"""Distributed Trainium2 Bass kernel for 16-head attention.

Reference op: B=2, S=2048, D=1024, H=16 multi-head attention with an
elementwise 0/1 mask, computed as
    out = softmax(mask((q Wq^T)(k Wk^T)^T / sqrt(64))) (v Wv^T) Wo^T

Sharding over 8 NeuronCores: core c handles batch c//4 and head group
c%4 (4 heads = 256 channels). Attention is computed fully locally in a
"dual" layout (scores transposed, [k, q]); the context is exchanged
with one small AllGather per 512-query tile inside each 4-core batch
group, and the output projection is split along the OUTPUT feature dim
(each core holds a 256-column slice of Wo^T), so the host-side unshard
is a pure concatenation.

v2 changes vs baseline:
  - all input DMAs (qT/kT/mask0/vT + weights) issued upfront so the
    projection phase is compute- not latency-bound
  - softmax normalization broadcast moved from a PE ones-matmul to
    gpsimd.partition_broadcast (Pool engine is otherwise idle), freeing
    PE cycles and a PSUM bank
  - outproj PSUM pool deepened to 4 x [128,256]

Compute dtype bf16 (TensorE 1 cyc/row), accumulation f32 in PSUM.
"""

import sys

sys.path.insert(0, "/opt/trn_rl_repo")

import numpy as np
import ml_dtypes

BF16 = ml_dtypes.bfloat16
FP8 = ml_dtypes.float8_e4m3

B = 2
S = 2048
DM = 1024
DL = 256  # d-model slice per core (4 heads)
HL = 4  # heads per core
DK = 64
P = 128
QT_N = 4  # query tiles of 512
QTS = 512
KC = 16  # key chunks of 128
MC = 8  # contraction chunks of 128 over d_model
GROUPS = [[0, 1, 2, 3], [4, 5, 6, 7]]

_cached = {}


def _build():
    import concourse.bass as bass
    import concourse.mybir as mybir
    from concourse import bacc
    from concourse.tile import TileContext

    fp32 = mybir.dt.float32
    bf16 = mybir.dt.bfloat16
    fp8 = mybir.dt.float8e4
    DR = mybir.MatmulPerfMode.DoubleRow

    nc = bacc.Bacc(num_devices=8)

    qT = nc.dram_tensor("qT", [DM, S], bf16, kind="ExternalInput")
    kT = nc.dram_tensor("kT", [DM, S], bf16, kind="ExternalInput")
    vT = nc.dram_tensor("vT", [DM, S], bf16, kind="ExternalInput")
    maskT = nc.dram_tensor("maskT", [S, S], bf16, kind="ExternalInput")
    wq = nc.dram_tensor("wq", [DM, DL], bf16, kind="ExternalInput")
    wk = nc.dram_tensor("wk", [DM, DL], bf16, kind="ExternalInput")
    wv = nc.dram_tensor("wv", [DM, DL], bf16, kind="ExternalInput")
    wo = nc.dram_tensor("wo", [DM, DL], bf16, kind="ExternalInput")
    y = nc.dram_tensor("y", [S, DL], fp32, kind="ExternalOutput")

    cc_in = [
        [
            nc.dram_tensor(f"cc_in{t}_{p}", [P, QTS], bf16, kind="Internal")
            for p in range(2)
        ]
        for t in range(QT_N)
    ]
    cc_out = [
        [
            nc.dram_tensor(f"cc_out{t}_{p}", [4 * P, QTS], bf16, kind="Internal")
            for p in range(2)
        ]
        for t in range(QT_N)
    ]

    with TileContext(nc) as tc:
        with (
            tc.tile_pool(name="xT", bufs=16) as xT_pool,
            tc.tile_pool(name="w", bufs=32) as w_pool,
            tc.tile_pool(name="qkt", bufs=2) as qkt_pool,
            tc.tile_pool(name="vext", bufs=16) as vext_pool,
            tc.tile_pool(name="mask", bufs=2) as mask_pool,
            tc.tile_pool(name="attn", bufs=4) as attn_pool,
            tc.tile_pool(name="sm", bufs=3) as sm_pool,
            tc.tile_pool(name="ctxn", bufs=2) as ctxn_pool,
            tc.tile_pool(name="ctxg", bufs=4) as ctxg_pool,
            tc.tile_pool(name="ysb", bufs=2) as y_pool,
            tc.tile_pool(name="ps_big", bufs=2, space="PSUM") as ps_big,
            tc.tile_pool(name="ps_acc", bufs=2, space="PSUM") as ps_acc,
            tc.tile_pool(name="ps_out", bufs=2, space="PSUM") as ps_out,
        ):
            # ---- upfront input DMAs -----------------------------------------
            def load_x(x_dram):
                tiles = []
                for m in range(MC):
                    t_ = xT_pool.tile([P, S], bf16, tag="xT", name=f"x{m}")
                    for h in range(4):
                        nc.sync.dma_start(
                            t_[:, QTS * h : QTS * (h + 1)],
                            x_dram[P * m : P * (m + 1), QTS * h : QTS * (h + 1)],
                        )
                    tiles.append(t_)
                return tiles

            def load_w(dram, nm):
                tiles = []
                for m in range(MC):
                    t_ = w_pool.tile([P, DL], bf16, tag="w", name=f"w{nm}{m}")
                    nc.sync.dma_start(t_[:], dram[P * m : P * (m + 1), :])
                    tiles.append(t_)
                return tiles

            def load_mask(t):
                # one DMA per key chunk so the 2MB tile sprays across DMA
                # queues instead of trickling through one
                mt_ = mask_pool.tile(
                    [P, KC * QTS], bf16, tag="mask", name=f"mask{t}"
                )
                for kc in range(KC):
                    nc.sync.dma_start(
                        mt_[:, QTS * kc : QTS * (kc + 1)],
                        maskT[P * kc : P * (kc + 1), QTS * t : QTS * (t + 1)],
                    )
                return mt_

            wq_sb = load_w(wq, "q")
            q_sb = load_x(qT)
            wv_sb = load_w(wv, "v")
            v_sb = load_x(vT)
            wk_sb = load_w(wk, "k")
            k_sb = load_x(kT)
            mts = {0: load_mask(0)}
            wo_sb = load_w(wo, "o")

            # ---- Q/K projections: out QT/KT [256, 2048] as 2 tiles [128,2048]
            def proj_T(x_sb, w_sb, tag):
                out_tiles = []
                for dt in range(2):
                    ot = qkt_pool.tile([P, S], bf16, tag=tag)
                    for st in range(2):
                        ps = ps_big.tile([P, 1024], fp32, tag="big")
                        for m in range(MC):
                            for sh in range(2):
                                nc.tensor.matmul(
                                    ps[:, QTS * sh : QTS * (sh + 1)],
                                    w_sb[m][:, P * dt : P * (dt + 1)],
                                    x_sb[m][
                                        :,
                                        1024 * st + QTS * sh : 1024 * st + QTS * (sh + 1),
                                    ],
                                    start=(m == 0),
                                    stop=(m == MC - 1),
                                )
                        nc.vector.tensor_copy(
                            ot[:, 1024 * st : 1024 * (st + 1)], ps[:]
                        )
                    out_tiles.append(ot)
                return out_tiles

            QT_sb = proj_T(q_sb, wq_sb, "QT")

            # Zero-padded per-head KT tiles so the scores matmuls run in full
            # 128x128 PE mode. KTz[pair][h01] has the head's 64 KT rows in
            # their original partitions and zeros in the other 64; the full QT
            # tile streams as rhs (zero weight rows kill the other head).
            KTz = [
                [
                    qkt_pool.tile(
                        [P, S], bf16, tag="KTz", bufs=4, name=f"ktz{dt}_{h01}"
                    )
                    for h01 in range(2)
                ]
                for dt in range(2)
            ]
            for dt in range(2):
                nc.vector.memset(KTz[dt][0][:], 0.0)
                nc.vector.memset(KTz[dt][1][:], 0.0)

            # ---- V projection -> V_ext tiles [128, 4*65] ([V_h | 1] blocks)
            # all-ones lhsT for the denominator broadcast matmul (full 128
            # contraction; the srow rhs is zero except its denominator row)
            ones_lhs = sm_pool.tile([P, P], bf16, tag="ones")
            nc.vector.memset(ones_lhs[:], 1.0)

            vext = []
            for st in range(KC):
                ps = ps_acc.tile([P, QTS], fp32, tag="acc", name=f"vp{st}")
                for m in range(MC):
                    nc.tensor.matmul(
                        ps[:, 0:DL],
                        v_sb[m][:, P * st : P * (st + 1)],
                        wv_sb[m][:],
                        start=(m == 0),
                        stop=(m == MC - 1),
                    )
                ve = vext_pool.tile([P, HL * (DK + 1)], bf16, tag="vext", name=f"ve{st}")
                nc.vector.memset(ve[:], 1.0)
                for h in range(HL):
                    nc.vector.tensor_copy(
                        ve[:, 65 * h : 65 * h + DK],
                        ps[:, DK * h : DK * (h + 1)],
                    )
                vext.append(ve)

            def kproj_chunk(dt, c, pool, tag):
                # one 512-col chunk of the K projection for head-pair dt,
                # written into the zero-padded KTz tiles
                ps = pool.tile([P, QTS], fp32, tag=tag, name=f"kp{dt}_{c}")
                for m in range(MC):
                    nc.tensor.matmul(
                        ps[:],
                        wk_sb[m][:, P * dt : P * (dt + 1)],
                        k_sb[m][:, QTS * c : QTS * (c + 1)],
                        start=(m == 0),
                        stop=(m == MC - 1),
                    )
                cols = slice(QTS * c, QTS * (c + 1))
                nc.vector.tensor_copy(KTz[dt][0][0:DK, cols], ps[0:DK, :])
                nc.vector.tensor_copy(KTz[dt][1][DK:P, cols], ps[DK:P, :])

            for dt in range(2):
                for c in range(4):
                    kproj_chunk(dt, c, ps_big, "big")
            k_fillers = []

            # ---- attention + exchange + output projection per query tile ----
            # The exchange readback + output projection for query tile t are
            # issued inside tile t+1's block so the AllGather latency hides
            # under the next tile's attention and never head-of-line-blocks
            # an engine queue.
            def do_readback(t, pairs=(0, 1)):
                ctxg = []
                for p in pairs:
                    cg = ctxg_pool.tile(
                        [P, 4 * QTS], bf16, tag="ctxg", name=f"cg{t}_{p}"
                    )
                    for i in range(4):
                        nc.sync.dma_start(
                            cg[:, QTS * i : QTS * (i + 1)],
                            cc_out[t][p][P * i : P * (i + 1), :],
                        )
                    ctxg.append(cg)
                return ctxg

            DCS = [0, 2, 4, 6, 1, 3, 5, 7]

            def outproj_steps(t, ctxg):
                # Generator of small out-proj work units (2 matmuls each) to
                # interleave into the next tile's attention stream, keeping
                # the PE queue stocked with always-ready work.
                state = {}

                def unit(qs, i0):
                    if qs not in state:
                        state[qs] = ps_out.tile(
                            [P, DL], fp32, tag="out", name=f"op{t}_{qs}"
                        )
                    op = state[qs]
                    for i in (i0, i0 + 1):
                        dc = DCS[i]
                        src = ctxg[dc % 2][
                            :,
                            QTS * (dc // 2) + P * qs : QTS * (dc // 2)
                            + P * (qs + 1),
                        ]
                        nc.tensor.matmul(
                            op[:],
                            src,
                            wo_sb[dc][:],
                            start=(i == 0),
                            stop=(i == MC - 1),
                        )
                    if i0 + 2 == MC:
                        ys = y_pool.tile(
                            [P, DL], fp32, tag="ysb", name=f"ys{t}_{qs}"
                        )
                        nc.vector.tensor_copy(ys[:], op[:])
                        r = QTS * t + P * qs
                        nc.sync.dma_start(y[r : r + P, :], ys[:])

                for qs in range(4):
                    for i0 in range(0, MC, 2):
                        yield lambda qs=qs, i0=i0: unit(qs, i0)

            def do_outproj(t, ctxg, qs_list=(0, 1, 2, 3)):
                steps = list(outproj_steps(t, ctxg))
                for st_ in steps:
                    st_()

            # ---- flat slot pipeline over (qtile, pair, group) ----------------
            # 64 scores/exp/mask slots; ctx accumulation trails by 3 slots and
            # flows continuously across pair and qtile boundaries so the PE
            # stream never thins out. attnT tiles are rolling 8-chunk buffers.
            ATD = 8
            at_store = {}
            cp_store = {}
            rolling_cols = ATD * QTS

            def emit_scores(u, grp):
                t, pair = divmod(u, 2)
                if grp == 0:
                    at_store[u] = {
                        h01: attn_pool.tile(
                            [P, rolling_cols], bf16, tag="attn",
                            name=f"at{u}_{h01}",
                        )
                        for h01 in range(2)
                    }
                    if pair == 0 and t + 1 < QT_N:
                        mts[t + 1] = load_mask(t + 1)
                at = at_store[u]
                mt = mts[t]
                sp = {}
                for h01 in range(2):
                    sp[h01] = ps_big.tile(
                        [P, 1024], fp32, tag="big", name=f"sp{u}_{grp}_{h01}"
                    )
                for j in range(2):
                    kc = 2 * grp + j
                    for h01 in range(2):
                        nc.tensor.matmul(
                            sp[h01][:, QTS * j : QTS * (j + 1)],
                            KTz[pair][h01][:, P * kc : P * (kc + 1)],
                            QT_sb[pair][:, QTS * t : QTS * (t + 1)],
                            start=True,
                            stop=True,
                        )
                roff = (2 * grp % ATD) * QTS
                rsl = slice(roff, roff + 1024)
                gsl = slice(1024 * grp, 1024 * (grp + 1))
                for h01 in range(2):
                    nc.scalar.activation(
                        at[h01][:, rsl],
                        sp[h01][:],
                        mybir.ActivationFunctionType.Exp,
                    )
                    nc.vector.tensor_mul(at[h01][:, rsl], at[h01][:, rsl], mt[:, gsl])

            def emit_ctx(u, grp):
                t, pair = divmod(u, 2)
                if grp == 0:
                    cp_store[u] = {
                        h01: ps_acc.tile(
                            [P, QTS], fp32, tag="acc", name=f"cp{u}_{h01}"
                        )
                        for h01 in range(2)
                    }
                at = at_store[u]
                cp = cp_store[u]
                for j in range(2):
                    kc = 2 * grp + j
                    roff = (kc % ATD) * QTS
                    for h01 in range(2):
                        h = 2 * pair + h01
                        nc.tensor.matmul(
                            cp[h01][0 : DK + 1, :],
                            vext[kc][:, 65 * h : 65 * h + DK + 1],
                            at[h01][:, roff : roff + QTS],
                            start=(kc == 0),
                            stop=(kc == KC - 1),
                        )

            def emit_norm(u):
                t, pair = divmod(u, 2)
                cp = cp_store[u]
                for h01 in range(2):
                    # srow is zero except the denominator row, so the all-ones
                    # full-128 matmul broadcasts that row to all partitions
                    # without switching the PE into tiled mode
                    srow = sm_pool.tile(
                        [P, QTS], bf16, tag="srow", name=f"srow{u}_{h01}"
                    )
                    nc.vector.memset(srow[:], 0.0)
                    nc.vector.tensor_copy(
                        srow[DK : DK + 1, :], cp[h01][DK : DK + 1, :]
                    )
                    bc = ps_out.tile(
                        [P, QTS], fp32, tag="out", name=f"bc{u}_{h01}"
                    )
                    nc.tensor.matmul(
                        bc[:],
                        ones_lhs[:],
                        srow[:],
                        start=True,
                        stop=True,
                    )
                    recipb = sm_pool.tile(
                        [P, QTS], fp32, tag="recipb", name=f"recipb{u}_{h01}"
                    )
                    nc.vector.reciprocal_approx_fast(out=recipb[:], in_=bc[:])
                    cn = ctxn_pool.tile(
                        [DK, QTS], bf16, tag="ctxn", name=f"cn{u}_{h01}"
                    )
                    nc.vector.tensor_mul(
                        cn[:], cp[h01][0:DK, :], recipb[0:DK, :]
                    )
                    nc.sync.dma_start(
                        cc_in[t][pair][DK * h01 : DK * (h01 + 1), :], cn[:]
                    )
                nc.gpsimd.collective_compute(
                    "AllGather",
                    mybir.AluOpType.bypass,
                    replica_groups=GROUPS,
                    ins=[cc_in[t][pair][:]],
                    outs=[cc_out[t][pair][:]],
                )
                del cp_store[u], at_store[u]

            op_steps = []
            NSLOT = 8 * 2 * QT_N
            ctx_done = 0  # flat index of next ctx slot to emit
            cur_slot = [0]

            def emit_ctx_flat(lag):
                ul, gl = divmod(lag, 8)
                emit_ctx(ul, gl)
                if gl == 7:
                    emit_norm(ul)
                    tl, pl = divmod(ul, 2)
                    if pl == 1 and tl < QT_N - 1:
                        ctxg_t = do_readback(tl)
                        # hold outproj matmuls out of the in-order PE queue
                        # until the AllGather has had time to land, else they
                        # head-of-line-block the attention stream
                        rel = cur_slot[0] + 10
                        op_steps.extend(
                            (rel, st) for st in outproj_steps(tl, ctxg_t)
                        )

            for i in range(NSLOT):
                cur_slot[0] = i
                u, grp = divmod(i, 8)
                emit_scores(u, grp)
                popped = 0
                while op_steps and popped < 4 and op_steps[0][0] <= i:
                    op_steps.pop(0)[1]()
                    popped += 1
                # trail by 3 slots; in the final unit converge to lag 1 so the
                # last exchanges issue as early as possible
                target = i - 3 if i < NSLOT - 8 else i - 1
                while ctx_done <= target and ctx_done < NSLOT:
                    emit_ctx_flat(ctx_done)
                    ctx_done += 1
            while ctx_done < NSLOT:
                emit_ctx_flat(ctx_done)
                ctx_done += 1
            ctxg_last = do_readback(QT_N - 1)
            for _, st_ in op_steps:
                st_()
            do_outproj(QT_N - 1, ctxg_last)

    nc.compile()
    return nc


def _get_nc():
    if "nc" not in _cached:
        _cached["nc"] = _build()
    return _cached["nc"]


def _shard_inputs(q, k, v, mask, w_q, w_k, w_v, w_o):
    in_maps = []
    scale = 1.0 / np.sqrt(DK)
    wqT = (w_q.astype(np.float64) * scale).astype(np.float32).T  # [DM, DM]
    wkT = w_k.T
    wvT = w_v.T
    woT = w_o.T
    for c in range(8):
        b, g = c // 4, c % 4
        sl = slice(DL * g, DL * (g + 1))
        in_maps.append(
            {
                "qT": np.ascontiguousarray(q[b].T).astype(BF16),
                "kT": np.ascontiguousarray(k[b].T).astype(BF16),
                "vT": np.ascontiguousarray(v[b].T).astype(BF16),
                "maskT": np.ascontiguousarray(mask[b].T).astype(BF16),
                "wq": np.ascontiguousarray(wqT[:, sl]).astype(BF16),
                "wk": np.ascontiguousarray(wkT[:, sl]).astype(BF16),
                "wv": np.ascontiguousarray(wvT[:, sl]).astype(BF16),
                "wo": np.ascontiguousarray(woT[:, sl]).astype(BF16),
            }
        )
    return in_maps


def kernel(q, k, v, mask, w_q, w_k, w_v, w_o, _trace=False, _tmpdir=None):
    from concourse import bass_utils

    nc = _get_nc()
    in_maps = _shard_inputs(q, k, v, mask, w_q, w_k, w_v, w_o)
    res = bass_utils.run_bass_kernel_spmd(
        nc,
        in_maps,
        core_ids=list(range(8)),
        trace=_trace,
        tmpdir=_tmpdir,
    )
    out = np.empty((B, S, DM), dtype=np.float32)
    for c in range(8):
        b, g = c // 4, c % 4
        out[b, :, DL * g : DL * (g + 1)] = res.results[c]["y"]
    if _trace:
        _cached["last_exec_time_ns"] = res.exec_time_ns
        _cached["last_results"] = res
    return out


# revision 45
# speedup vs baseline: 1.0634x; 1.0634x over previous
"""Distributed Trainium2 Bass kernel for 16-head attention.

Reference op: B=2, S=2048, D=1024, H=16 multi-head attention with an
elementwise 0/1 mask, computed as
    out = softmax(mask((q Wq^T)(k Wk^T)^T / sqrt(64))) (v Wv^T) Wo^T

Sharding over 8 NeuronCores: core c handles batch c//4 and head group
c%4 (4 heads = 256 channels). Attention is computed fully locally in a
"dual" layout (scores transposed, [k, q]); the context is exchanged
with one small AllGather per 512-query tile inside each 4-core batch
group, and the output projection is split along the OUTPUT feature dim
(each core holds a 256-column slice of Wo^T), so the host-side unshard
is a pure concatenation.

v2 changes vs baseline:
  - all input DMAs (qT/kT/mask0/vT + weights) issued upfront so the
    projection phase is compute- not latency-bound
  - softmax normalization broadcast moved from a PE ones-matmul to
    gpsimd.partition_broadcast (Pool engine is otherwise idle), freeing
    PE cycles and a PSUM bank
  - outproj PSUM pool deepened to 4 x [128,256]

Compute dtype bf16 (TensorE 1 cyc/row), accumulation f32 in PSUM.
"""

import sys

sys.path.insert(0, "/opt/trn_rl_repo")

import numpy as np
import ml_dtypes

BF16 = ml_dtypes.bfloat16
FP8 = ml_dtypes.float8_e4m3

B = 2
S = 2048
DM = 1024
DL = 256  # d-model slice per core (4 heads)
HL = 4  # heads per core
DK = 64
P = 128
QT_N = 4  # query tiles of 512
QTS = 512
KC = 16  # key chunks of 128
MC = 8  # contraction chunks of 128 over d_model
GROUPS = [[0, 1, 2, 3], [4, 5, 6, 7]]

_cached = {}


def _build():
    import concourse.bass as bass
    import concourse.mybir as mybir
    from concourse import bacc
    from concourse.tile import TileContext

    fp32 = mybir.dt.float32
    bf16 = mybir.dt.bfloat16
    fp8 = mybir.dt.float8e4
    DR = mybir.MatmulPerfMode.DoubleRow

    nc = bacc.Bacc(num_devices=8)

    qT = nc.dram_tensor("qT", [DM, S], bf16, kind="ExternalInput")
    kT = nc.dram_tensor("kT", [DM, S], bf16, kind="ExternalInput")
    vT = nc.dram_tensor("vT", [DM, S], bf16, kind="ExternalInput")
    maskT = nc.dram_tensor("maskT", [S, S], bf16, kind="ExternalInput")
    wq = nc.dram_tensor("wq", [DM, DL], bf16, kind="ExternalInput")
    wk = nc.dram_tensor("wk", [DM, DL], bf16, kind="ExternalInput")
    wv = nc.dram_tensor("wv", [DM, DL], bf16, kind="ExternalInput")
    wo = nc.dram_tensor("wo", [DM, DL], bf16, kind="ExternalInput")
    y = nc.dram_tensor("y", [S, DL], fp32, kind="ExternalOutput")

    cc_in = [
        [
            nc.dram_tensor(f"cc_in{t}_{p}", [P, QTS], bf16, kind="Internal")
            for p in range(2)
        ]
        for t in range(QT_N)
    ]
    cc_out = [
        [
            nc.dram_tensor(f"cc_out{t}_{p}", [4 * P, QTS], bf16, kind="Internal")
            for p in range(2)
        ]
        for t in range(QT_N)
    ]

    with TileContext(nc) as tc:
        with (
            tc.tile_pool(name="xT", bufs=16) as xT_pool,
            tc.tile_pool(name="w", bufs=32) as w_pool,
            tc.tile_pool(name="qkt", bufs=2) as qkt_pool,
            tc.tile_pool(name="vext", bufs=16) as vext_pool,
            tc.tile_pool(name="mask", bufs=2) as mask_pool,
            tc.tile_pool(name="attn", bufs=4) as attn_pool,
            tc.tile_pool(name="sm", bufs=3) as sm_pool,
            tc.tile_pool(name="ctxn", bufs=2) as ctxn_pool,
            tc.tile_pool(name="ctxg", bufs=4) as ctxg_pool,
            tc.tile_pool(name="ysb", bufs=2) as y_pool,
            tc.tile_pool(name="ps_big", bufs=2, space="PSUM") as ps_big,
            tc.tile_pool(name="ps_acc", bufs=2, space="PSUM") as ps_acc,
            tc.tile_pool(name="ps_out", bufs=2, space="PSUM") as ps_out,
        ):
            # ---- upfront input DMAs -----------------------------------------
            def load_x(x_dram):
                tiles = []
                for m in range(MC):
                    t_ = xT_pool.tile([P, S], bf16, tag="xT", name=f"x{m}")
                    for h in range(4):
                        nc.sync.dma_start(
                            t_[:, QTS * h : QTS * (h + 1)],
                            x_dram[P * m : P * (m + 1), QTS * h : QTS * (h + 1)],
                        )
                    tiles.append(t_)
                return tiles

            def load_w(dram, nm):
                tiles = []
                for m in range(MC):
                    t_ = w_pool.tile([P, DL], bf16, tag="w", name=f"w{nm}{m}")
                    nc.sync.dma_start(t_[:], dram[P * m : P * (m + 1), :])
                    tiles.append(t_)
                return tiles

            def load_mask(t):
                # one DMA per key chunk so the 2MB tile sprays across DMA
                # queues instead of trickling through one
                mt_ = mask_pool.tile(
                    [P, KC * QTS], bf16, tag="mask", name=f"mask{t}"
                )
                for kc in range(KC):
                    nc.sync.dma_start(
                        mt_[:, QTS * kc : QTS * (kc + 1)],
                        maskT[P * kc : P * (kc + 1), QTS * t : QTS * (t + 1)],
                    )
                return mt_

            wq_sb = load_w(wq, "q")
            q_sb = load_x(qT)
            wk_sb = load_w(wk, "k")
            k_sb = load_x(kT)
            wv_sb = load_w(wv, "v")
            v_sb = load_x(vT)
            mts = {0: load_mask(0)}
            wo_sb = load_w(wo, "o")

            # ---- Q/K projections: out QT/KT [256, 2048] as 2 tiles [128,2048]
            def proj_T(x_sb, w_sb, tag):
                out_tiles = []
                for dt in range(2):
                    ot = qkt_pool.tile([P, S], bf16, tag=tag)
                    for st in range(2):
                        ps = ps_big.tile([P, 1024], fp32, tag="big")
                        for m in range(MC):
                            for sh in range(2):
                                nc.tensor.matmul(
                                    ps[:, QTS * sh : QTS * (sh + 1)],
                                    w_sb[m][:, P * dt : P * (dt + 1)],
                                    x_sb[m][
                                        :,
                                        1024 * st + QTS * sh : 1024 * st + QTS * (sh + 1),
                                    ],
                                    start=(m == 0),
                                    stop=(m == MC - 1),
                                )
                        nc.vector.tensor_copy(
                            ot[:, 1024 * st : 1024 * (st + 1)], ps[:]
                        )
                    out_tiles.append(ot)
                return out_tiles

            QT_sb = proj_T(q_sb, wq_sb, "QT")

            # Zero-padded per-head KT tiles so the scores matmuls run in full
            # 128x128 PE mode. KTz[pair][h01] has the head's 64 KT rows in
            # their original partitions and zeros in the other 64; the full QT
            # tile streams as rhs (zero weight rows kill the other head).
            KTz = [
                [
                    qkt_pool.tile(
                        [P, S], bf16, tag="KTz", bufs=4, name=f"ktz{dt}_{h01}"
                    )
                    for h01 in range(2)
                ]
                for dt in range(2)
            ]
            for dt in range(2):
                nc.vector.memset(KTz[dt][0][:], 0.0)
                nc.vector.memset(KTz[dt][1][:], 0.0)

            # ---- V projection -> V_ext tiles [128, 4*65] ([V_h | 1] blocks)
            # all-ones lhsT for the denominator broadcast matmul (full 128
            # contraction; the srow rhs is zero except its denominator row)
            ones_lhs = sm_pool.tile([P, P], bf16, tag="ones")
            nc.vector.memset(ones_lhs[:], 1.0)

            vext = []
            for st in range(KC):
                ps = ps_acc.tile([P, QTS], fp32, tag="acc", name=f"vp{st}")
                for m in range(MC):
                    nc.tensor.matmul(
                        ps[:, 0:DL],
                        v_sb[m][:, P * st : P * (st + 1)],
                        wv_sb[m][:],
                        start=(m == 0),
                        stop=(m == MC - 1),
                    )
                ve = vext_pool.tile([P, HL * (DK + 1)], bf16, tag="vext", name=f"ve{st}")
                nc.vector.memset(ve[:], 1.0)
                for h in range(HL):
                    nc.vector.tensor_copy(
                        ve[:, 65 * h : 65 * h + DK],
                        ps[:, DK * h : DK * (h + 1)],
                    )
                vext.append(ve)

            def kproj_chunk(dt, c, pool, tag):
                # one 512-col chunk of the K projection for head-pair dt,
                # written into the zero-padded KTz tiles
                ps = pool.tile([P, QTS], fp32, tag=tag, name=f"kp{dt}_{c}")
                for m in range(MC):
                    nc.tensor.matmul(
                        ps[:],
                        wk_sb[m][:, P * dt : P * (dt + 1)],
                        k_sb[m][:, QTS * c : QTS * (c + 1)],
                        start=(m == 0),
                        stop=(m == MC - 1),
                    )
                cols = slice(QTS * c, QTS * (c + 1))
                nc.vector.tensor_copy(KTz[dt][0][0:DK, cols], ps[0:DK, :])
                nc.vector.tensor_copy(KTz[dt][1][DK:P, cols], ps[DK:P, :])

            for dt in range(2):
                for c in range(4):
                    kproj_chunk(dt, c, ps_big, "big")
            k_fillers = []

            # ---- attention + exchange + output projection per query tile ----
            # The exchange readback + output projection for query tile t are
            # issued inside tile t+1's block so the AllGather latency hides
            # under the next tile's attention and never head-of-line-blocks
            # an engine queue.
            def do_readback(t, pairs=(0, 1)):
                ctxg = []
                for p in pairs:
                    cg = ctxg_pool.tile(
                        [P, 4 * QTS], bf16, tag="ctxg", name=f"cg{t}_{p}"
                    )
                    for i in range(4):
                        nc.sync.dma_start(
                            cg[:, QTS * i : QTS * (i + 1)],
                            cc_out[t][p][P * i : P * (i + 1), :],
                        )
                    ctxg.append(cg)
                return ctxg

            DCS = [0, 2, 4, 6, 1, 3, 5, 7]

            def outproj_steps(t, ctxg):
                # Generator of small out-proj work units (2 matmuls each) to
                # interleave into the next tile's attention stream, keeping
                # the PE queue stocked with always-ready work.
                state = {}

                def unit(qs, i0):
                    if qs not in state:
                        state[qs] = ps_out.tile(
                            [P, DL], fp32, tag="out", name=f"op{t}_{qs}"
                        )
                    op = state[qs]
                    for i in (i0, i0 + 1):
                        dc = DCS[i]
                        src = ctxg[dc % 2][
                            :,
                            QTS * (dc // 2) + P * qs : QTS * (dc // 2)
                            + P * (qs + 1),
                        ]
                        nc.tensor.matmul(
                            op[:],
                            src,
                            wo_sb[dc][:],
                            start=(i == 0),
                            stop=(i == MC - 1),
                        )
                    if i0 + 2 == MC:
                        ys = y_pool.tile(
                            [P, DL], fp32, tag="ysb", name=f"ys{t}_{qs}"
                        )
                        nc.vector.tensor_copy(ys[:], op[:])
                        r = QTS * t + P * qs
                        nc.sync.dma_start(y[r : r + P, :], ys[:])

                for qs in range(4):
                    for i0 in range(0, MC, 2):
                        yield lambda qs=qs, i0=i0: unit(qs, i0)

            def do_outproj(t, ctxg, qs_list=(0, 1, 2, 3)):
                steps = list(outproj_steps(t, ctxg))
                for st_ in steps:
                    st_()

            # ---- flat slot pipeline over (qtile, pair, group) ----------------
            # 64 scores/exp/mask slots; ctx accumulation trails by 3 slots and
            # flows continuously across pair and qtile boundaries so the PE
            # stream never thins out. attnT tiles are rolling 8-chunk buffers.
            ATD = 8
            at_store = {}
            cp_store = {}
            rolling_cols = ATD * QTS

            def emit_scores(u, grp):
                t, pair = divmod(u, 2)
                if grp == 0:
                    at_store[u] = {
                        h01: attn_pool.tile(
                            [P, rolling_cols], bf16, tag="attn",
                            name=f"at{u}_{h01}",
                        )
                        for h01 in range(2)
                    }
                    if pair == 0 and t + 1 < QT_N:
                        mts[t + 1] = load_mask(t + 1)
                at = at_store[u]
                mt = mts[t]
                sp = {}
                for h01 in range(2):
                    sp[h01] = ps_big.tile(
                        [P, 1024], fp32, tag="big", name=f"sp{u}_{grp}_{h01}"
                    )
                for j in range(2):
                    kc = 2 * grp + j
                    for h01 in range(2):
                        nc.tensor.matmul(
                            sp[h01][:, QTS * j : QTS * (j + 1)],
                            KTz[pair][h01][:, P * kc : P * (kc + 1)],
                            QT_sb[pair][:, QTS * t : QTS * (t + 1)],
                            start=True,
                            stop=True,
                        )
                roff = (2 * grp % ATD) * QTS
                rsl = slice(roff, roff + 1024)
                gsl = slice(1024 * grp, 1024 * (grp + 1))
                for h01 in range(2):
                    nc.scalar.activation(
                        at[h01][:, rsl],
                        sp[h01][:],
                        mybir.ActivationFunctionType.Exp,
                    )
                    nc.vector.tensor_mul(at[h01][:, rsl], at[h01][:, rsl], mt[:, gsl])

            def emit_ctx(u, grp):
                t, pair = divmod(u, 2)
                if grp == 0:
                    cp_store[u] = {
                        h01: ps_acc.tile(
                            [P, QTS], fp32, tag="acc", name=f"cp{u}_{h01}"
                        )
                        for h01 in range(2)
                    }
                at = at_store[u]
                cp = cp_store[u]
                for j in range(2):
                    kc = 2 * grp + j
                    roff = (kc % ATD) * QTS
                    for h01 in range(2):
                        h = 2 * pair + h01
                        nc.tensor.matmul(
                            cp[h01][0 : DK + 1, :],
                            vext[kc][:, 65 * h : 65 * h + DK + 1],
                            at[h01][:, roff : roff + QTS],
                            start=(kc == 0),
                            stop=(kc == KC - 1),
                        )

            def emit_norm(u):
                t, pair = divmod(u, 2)
                cp = cp_store[u]
                for h01 in range(2):
                    # srow is zero except the denominator row, so the all-ones
                    # full-128 matmul broadcasts that row to all partitions
                    # without switching the PE into tiled mode
                    srow = sm_pool.tile(
                        [P, QTS], bf16, tag="srow", name=f"srow{u}_{h01}"
                    )
                    nc.vector.memset(srow[:], 0.0)
                    nc.vector.tensor_copy(
                        srow[DK : DK + 1, :], cp[h01][DK : DK + 1, :]
                    )
                    bc = ps_out.tile(
                        [P, QTS], fp32, tag="out", name=f"bc{u}_{h01}"
                    )
                    nc.tensor.matmul(
                        bc[:],
                        ones_lhs[:],
                        srow[:],
                        start=True,
                        stop=True,
                    )
                    recipb = sm_pool.tile(
                        [P, QTS], fp32, tag="recipb", name=f"recipb{u}_{h01}"
                    )
                    nc.vector.reciprocal_approx_fast(out=recipb[:], in_=bc[:])
                    cn = ctxn_pool.tile(
                        [DK, QTS], bf16, tag="ctxn", name=f"cn{u}_{h01}"
                    )
                    nc.vector.tensor_mul(
                        cn[:], cp[h01][0:DK, :], recipb[0:DK, :]
                    )
                    nc.sync.dma_start(
                        cc_in[t][pair][DK * h01 : DK * (h01 + 1), :], cn[:]
                    )
                nc.gpsimd.collective_compute(
                    "AllGather",
                    mybir.AluOpType.bypass,
                    replica_groups=GROUPS,
                    ins=[cc_in[t][pair][:]],
                    outs=[cc_out[t][pair][:]],
                )
                del cp_store[u], at_store[u]

            op_steps = []
            NSLOT = 8 * 2 * QT_N
            ctx_done = 0  # flat index of next ctx slot to emit
            cur_slot = [0]

            def emit_ctx_flat(lag):
                ul, gl = divmod(lag, 8)
                emit_ctx(ul, gl)
                if gl == 7:
                    emit_norm(ul)
                    tl, pl = divmod(ul, 2)
                    if pl == 1 and tl < QT_N - 1:
                        ctxg_t = do_readback(tl)
                        # hold outproj matmuls out of the in-order PE queue
                        # until the AllGather has had time to land, else they
                        # head-of-line-block the attention stream
                        rel = cur_slot[0] + 10
                        op_steps.extend(
                            (rel, st) for st in outproj_steps(tl, ctxg_t)
                        )

            for i in range(NSLOT):
                cur_slot[0] = i
                u, grp = divmod(i, 8)
                emit_scores(u, grp)
                popped = 0
                while op_steps and popped < 4 and op_steps[0][0] <= i:
                    op_steps.pop(0)[1]()
                    popped += 1
                # trail by 3 slots; in the final unit converge to lag 1 so the
                # last exchanges issue as early as possible
                target = i - 3 if i < NSLOT - 8 else i - 1
                while ctx_done <= target and ctx_done < NSLOT:
                    emit_ctx_flat(ctx_done)
                    ctx_done += 1
            while ctx_done < NSLOT:
                emit_ctx_flat(ctx_done)
                ctx_done += 1
            ctxg_last = do_readback(QT_N - 1)
            for _, st_ in op_steps:
                st_()
            do_outproj(QT_N - 1, ctxg_last)

    nc.compile()
    return nc


def _get_nc():
    if "nc" not in _cached:
        _cached["nc"] = _build()
    return _cached["nc"]


def _shard_inputs(q, k, v, mask, w_q, w_k, w_v, w_o):
    in_maps = []
    scale = 1.0 / np.sqrt(DK)
    wqT = (w_q.astype(np.float64) * scale).astype(np.float32).T  # [DM, DM]
    wkT = w_k.T
    wvT = w_v.T
    woT = w_o.T
    for c in range(8):
        b, g = c // 4, c % 4
        sl = slice(DL * g, DL * (g + 1))
        in_maps.append(
            {
                "qT": np.ascontiguousarray(q[b].T).astype(BF16),
                "kT": np.ascontiguousarray(k[b].T).astype(BF16),
                "vT": np.ascontiguousarray(v[b].T).astype(BF16),
                "maskT": np.ascontiguousarray(mask[b].T).astype(BF16),
                "wq": np.ascontiguousarray(wqT[:, sl]).astype(BF16),
                "wk": np.ascontiguousarray(wkT[:, sl]).astype(BF16),
                "wv": np.ascontiguousarray(wvT[:, sl]).astype(BF16),
                "wo": np.ascontiguousarray(woT[:, sl]).astype(BF16),
            }
        )
    return in_maps


def kernel(q, k, v, mask, w_q, w_k, w_v, w_o, _trace=False, _tmpdir=None):
    from concourse import bass_utils

    nc = _get_nc()
    in_maps = _shard_inputs(q, k, v, mask, w_q, w_k, w_v, w_o)
    res = bass_utils.run_bass_kernel_spmd(
        nc,
        in_maps,
        core_ids=list(range(8)),
        trace=_trace,
        tmpdir=_tmpdir,
    )
    out = np.empty((B, S, DM), dtype=np.float32)
    for c in range(8):
        b, g = c // 4, c % 4
        out[b, :, DL * g : DL * (g + 1)] = res.results[c]["y"]
    if _trace:
        _cached["last_exec_time_ns"] = res.exec_time_ns
        _cached["last_results"] = res
    return out
